# revision 50
# baseline (speedup 1.0000x reference)
"""DepthNet cost-volume kernel for 8 Trainium2 NeuronCores (v3, cl-layout).

Partition layout: p = (cl, d) where cl = channel parity (2) and d = depth
(64); free dims are (c16, x) with c16 = channel-within-parity (16) and
x = full row width (160).  Channel ch = 2*c16 + cl.

Why: the 3x3x3 conv contracts (channel-pair, depth-band) on the PE with a
banded lhsT -- in this layout the variance ring rows ARE the conv rhs
(x-contiguous per (c16, slot)), so the former SBUF->SBUF relayout DMAs
(4 per row) disappear entirely.  x fully in the free dim also lets the
3rd bilinear column tap run only over the x-runs where the x-offset
straddles an integer (host-computed), instead of full width.

Per row r (16 output rows + 1 conv-halo row each side per core):
  S1 (x-interp, shared by 3 output rows): per (view, source row Y)
     Gx[c,x,d] = sum_i tentx_i(px(x,d)) * F[c,Y,x+sx+i]
     features replicated across the 64 d-partitions by one stride-0
     HWDGE DMA from a host-staged window tensor (all views in one DMA).
  S2 (y-interp): warped = sum_j tenty_j(py(x,y,d)) * Gx[y-1+j]
     vsum accumulated on PE (identity matmuls into PSUM), squares +
     vsq adds per engine knobs, variance = V*vsq - vsum^2 into an SBUF
     ring (1/V^2 folded into the conv band host-side).
  Conv (interleaved, one row behind): 3x3x3 conv = 144 PSUM-accumulated
     banded matmuls reading the variance ring directly; then softmax
     over depth and transposed store.
No cross-core communication.  Wide elementwise ops run fp16 (2x DVE).
"""
import os
import numpy as np
import concourse.bass as bass
import concourse.tile as tile
from concourse import bacc, mybir

F32 = mybir.dt.float32
F16 = mybir.dt.float16
OP = mybir.AluOpType
ACT = mybir.ActivationFunctionType

B, C, H, W, D, V = 1, 32, 128, 160, 64, 5
NCORES = 8
ROWS = H // NCORES          # output rows per core
RH = ROWS + 2               # with conv halo
SRC = RH + 2                # source rows touched per core
NQ = 16                     # conv channel-pair chunks
C16 = C // 2                # channels per parity
FW = 162                    # staged window width per view
CJ = C16 * W                # free size of one view-row block (2560)
CFW = C16 * FW
NVIEW = V - 1
WP = W + 2                  # padded variance width

_cache = {}
_GEOM = None                # (ncols tuple, runs) set by _host_prep

# engine split (v=DVE, g=Pool), tunable via env
_S1_ENG = os.environ.get("K_S1", "vvvv")   # per-view S1 x-interp
_S2_ENG = os.environ.get("K_S2", "vvvv")   # per-view S2 y-interp
_ACC_ENG = os.environ.get("K_ACC", "vv")   # vsum, vsq adds
_VAR_ENG = os.environ.get("K_VAR", "s")    # variance: 2=split, s=stt DVE
_TY_ENG = os.environ.get("K_TY", "a")      # tenty smalls: a=ACT, v=DVE
_VS_ENG = os.environ.get("K_VS", "p")      # vsum: p=PE psum, v/g=tt adds
_SQ_ENG = os.environ.get("K_SQ", "aaaaaa")  # squares: f0,4 views,m; a/v/g


def _build_program(nrep, geom):
    ncols, runs = geom
    nc = bacc.Bacc("TRN2", target_bir_lowering=False, debug=False,
                   num_devices=NCORES, num_swdge_queues=4)
    fwinall = nc.dram_tensor("fwinall", [SRC, 2, NVIEW * CFW], F16,
                             kind="ExternalInput")
    f0s = nc.dram_tensor("f0slab", [RH, 2, CJ], F16, kind="ExternalInput")
    a_all = nc.dram_tensor("a_all", [128, NVIEW * W], F32,
                           kind="ExternalInput")
    b1_all = nc.dram_tensor("b1_all", [128, NVIEW * W], F32,
                            kind="ExternalInput")
    txh = nc.dram_tensor("txall", [128, sum(ncols) * W], F16,
                         kind="ExternalInput")
    hmaskh = nc.dram_tensor("hmask", [128, 2 * RH], F32, kind="ExternalInput")
    bandh = nc.dram_tensor("band", [9 * NQ, 128, 64], F16,
                           kind="ExternalInput")
    identh = nc.dram_tensor("ident", [128, 128], F16, kind="ExternalInput")
    outh = nc.dram_tensor("out", [ROWS, D, W], F32, kind="ExternalOutput")

    with tile.TileContext(nc) as tc:
        import contextlib
        with contextlib.ExitStack() as ctx:
            const_p = ctx.enter_context(tc.tile_pool(name="const", bufs=1))
            aA = const_p.tile([128, NVIEW * W], F32)
            nc.sync.dma_start(aA[:], a_all.ap())
            bB = const_p.tile([128, NVIEW * W], F32)
            nc.sync.dma_start(bB[:], b1_all.ap())
            tx = const_p.tile([128, sum(ncols) * W], F16)
            nc.sync.dma_start(tx[:], txh.ap())
            hm = const_p.tile([128, 2 * RH], F32)
            nc.sync.dma_start(hm[:], hmaskh.ap())
            band = const_p.tile([128, 9 * NQ * 64], F16)
            nc.sync.dma_start(
                band[:],
                bass.AP(bandh, 0, [[64, 128], [8192, 9 * NQ], [1, 64]]))
            ident = const_p.tile([128, 128], F16)
            nc.sync.dma_start(ident[:], identh.ap())

            for rep in range(nrep):
                if rep > 0:
                    tc.strict_bb_all_engine_barrier()
                _emit_main(tc, nc, ncols, runs, fwinall, f0s, aA, bB, tx, hm,
                           band, ident, outh)
    nc.compile()
    return nc


def _eng(nc, ch):
    return nc.vector if ch == "v" else nc.gpsimd


_DMAQ_MAP = {"a": "scalar", "s": "sync", "p": "gpsimd"}


def _dmaq(nc, ch):
    return getattr(nc, _DMAQ_MAP[ch])


def _FR_DMA(nc):
    return _dmaq(nc, os.environ.get("K_FRQ", "s")[0])


def _F0_DMA(nc):
    return _dmaq(nc, os.environ.get("K_F0Q", "s")[0])


def _OUT_DMA(nc):
    return _dmaq(nc, os.environ.get("K_OUTQ", "s")[0])


def _emit_main(tc, nc, ncols, runs, fwinall, f0s, aA, bB, tx, hm, band,
               ident, outh):
    import contextlib
    txoff = [sum(ncols[:i]) * W for i in range(NVIEW)]
    with contextlib.ExitStack() as st:
        gx_p = st.enter_context(tc.tile_pool(name="gx", bufs=1))
        frep_p = st.enter_context(tc.tile_pool(name="frep", bufs=2))
        # tmpS1 is written+consumed back-to-back on the (in-order) DVE, so a
        # single buffer loses no overlap and frees 5KB/partition
        s1_p = st.enter_context(tc.tile_pool(name="s1", bufs=1))
        s2_p = st.enter_context(tc.tile_pool(name="s2", bufs=2))
        row_p = st.enter_context(tc.tile_pool(name="row", bufs=2))
        rw2_p = st.enter_context(tc.tile_pool(name="rw2", bufs=2))
        ty_p = st.enter_context(tc.tile_pool(name="ty", bufs=1))
        typ2 = st.enter_context(tc.tile_pool(name="typ2", bufs=2))
        m_p = st.enter_context(tc.tile_pool(name="msq", bufs=1))
        vr_p = st.enter_context(tc.tile_pool(name="vr", bufs=1))
        pp = st.enter_context(tc.tile_pool(name="cpsum", bufs=1, space="PSUM"))
        sp = st.enter_context(tc.tile_pool(name="soft", bufs=2))

        gslot, vslot = [], []
        for s in range(3):
            gs = gx_p.tile([128, NVIEW * CJ], F16, tag=f"gs{s}")
            gslot.append(gs)
            vs = vr_p.tile([128, C16 * WP], F16, tag=f"vr{s}")
            # zero the x-pad columns once; rows only ever write [1:W+1]
            nc.vector.memset(
                vs[:].rearrange("p (c x) -> p c x", x=WP)[:, :, 0:WP:WP - 1],
                0.0)
            vslot.append(vs)

        pv_p = st.enter_context(tc.tile_pool(name="vpsum", bufs=1,
                                             space="PSUM"))
        identf = ident
        ones64 = gx_p.tile([64, 64], F16, tag="ones64")
        nc.vector.memset(ones64[:], 1.0)

        def s1(ssi):
            # x-interp source row ssi (global Y = base-2+ssi) for all views
            s = ssi % 3
            fr = frep_p.tile([128, NVIEW * CFW], F16, tag="frall")
            nsp = int(os.environ.get("K_FRSPLIT", "1"))
            seg = NVIEW * CFW // nsp
            for si in range(nsp):
                in_ap = bass.AP(fwinall, ssi * (NVIEW * 2 * CFW) + si * seg,
                                [[NVIEW * CFW, 2], [0, 64], [1, seg]])
                _FR_DMA(nc).dma_start(fr[:, si * seg:(si + 1) * seg], in_ap)
            frv = fr[:].rearrange("p (v c w) -> p v c w", c=C16, w=FW)
            for vi in range(NVIEW):
                gsub = (gslot[s][:, vi * CJ:(vi + 1) * CJ]
                        .rearrange("p (c x) -> p c x", x=W))
                E = _eng(nc, _S1_ENG[vi])
                first = True
                for pos in range(ncols[vi]):
                    orig_i, rlist = runs[vi][pos]
                    for (xa, xb) in rlist:
                        txv = (tx[:, txoff[vi] + pos * W + xa:
                                  txoff[vi] + pos * W + xb]
                               .unsqueeze(1).broadcast_to([128, C16, xb - xa]))
                        fseg = frv[:, vi, :, orig_i + xa:orig_i + xb]
                        if first:
                            assert (xa, xb) == (0, W), \
                                "first S1 tap must cover full width"
                            E.tensor_tensor(gsub, txv, fseg, op=OP.mult)
                            first = False
                        else:
                            tm = s1_p.tile([128, CJ], F16, tag="tmpS1")
                            tmv = (tm[:].rearrange("p (c x) -> p c x", x=W)
                                   [:, :, xa:xb])
                            E.tensor_tensor(tmv, txv, fseg, op=OP.mult)
                            E.tensor_tensor(gsub[:, :, xa:xb],
                                            gsub[:, :, xa:xb], tmv, op=OP.add)

        pair = {}

        def conv_row(ro):
            cost = pp.tile([64, W], F32, tag="cost")
            first = True
            for dy in range(3):
                vsv = (vslot[(ro + dy - 1) % 3][:]
                       .rearrange("p (c x) -> p c x", x=WP))
                for dx in range(3):
                    t = dy * 3 + dx
                    for k in range(NQ):
                        rhs = vsv[:, k, dx:dx + W]
                        lhsT = band[:, (t * NQ + k) * 64:(t * NQ + k + 1) * 64]
                        last = (dy == 2 and dx == 2 and k == NQ - 1)
                        nc.tensor.matmul(cost[:], lhsT, rhs,
                                         start=first, stop=last)
                        first = False
            # transpose-free softmax over d (on partitions): |cost| is small
            # (conv of variance with ~0.05-scale weights), so exp needs no
            # max subtraction; depth-sum via all-ones PE matmul replicates
            # the denominator across all 64 partitions.  The tail (sum,
            # reciprocal, scale, store) is batched over row pairs to halve
            # the instruction and DMA-issue count.
            if not pair:
                e2 = sp.tile([64, 2 * W], F16, tag="e")
                pair["e"] = e2
                pair["ro"] = ro
                nc.scalar.activation(e2[:, 0:W], cost[:], ACT.Exp)
                return
            e2 = pair.pop("e")
            ro0 = pair.pop("ro")
            assert ro == ro0 + 1
            nc.scalar.activation(e2[:, W:2 * W], cost[:], ACT.Exp)
            ssum = pp.tile([64, 2 * W], F32, tag="ssum")
            nc.tensor.matmul(ssum[:], ones64[:], e2[:], start=True, stop=True)
            rinv = sp.tile([64, 2 * W], F16, tag="rinv")
            with nc.allow_low_precision(reason="softmax denom fp16 is ample"):
                nc.vector.reciprocal(rinv[:], ssum[:])
            prob = m_p.tile([64, 2 * W], F32, tag="prob")
            nc.vector.tensor_tensor(prob[:], e2[:], rinv[:], op=OP.mult)
            out_ap = bass.AP(outh, (ro0 - 1) * D * W,
                             [[W, 64], [D * W, 2], [1, W]])
            _OUT_DMA(nc).dma_start(out_ap, prob[:])

        def emit_ty(r):
            # tenty weights for the 3 source rows (masked for halo rows)
            pyr = ty_p.tile([128, NVIEW * W], F32, tag="pyr")
            nc.vector.scalar_tensor_tensor(pyr[:], bB[:], float(r), aA[:],
                                           op0=OP.mult, op1=OP.add)
            hmc = hm[:, r:r + 1]
            hmn = hm[:, RH + r:RH + r + 1]
            ty0 = typ2.tile([128, NVIEW * W], F16, tag="ty0")
            ty1 = typ2.tile([128, NVIEW * W], F16, tag="ty1")
            ty2 = typ2.tile([128, NVIEW * W], F16, tag="ty2")
            if _TY_ENG == "a":
                # masked tents on ACT: Relu(scale*x + bias), per-part scale
                nc.scalar.activation(ty0[:], pyr[:], ACT.Relu, scale=hmn)
                nc.scalar.activation(ty2[:], pyr[:], ACT.Relu, scale=hmc)
                ab = ty_p.tile([128, NVIEW * W], F16, tag="ng")
                nc.scalar.activation(ab[:], pyr[:], ACT.Abs)
                nc.scalar.activation(ty1[:], ab[:], ACT.Relu,
                                     scale=hmn, bias=hmc)
            else:
                nc.vector.tensor_scalar(ty0[:], pyr[:], hmn, 0.0,
                                        op0=OP.mult, op1=OP.max)
                nc.vector.tensor_scalar(ty2[:], pyr[:], hmc, 0.0,
                                        op0=OP.mult, op1=OP.max)
                ng = ty_p.tile([128, NVIEW * W], F32, tag="ng")
                nc.vector.tensor_scalar(ng[:], pyr[:], -1.0, None,
                                        op0=OP.mult)
                nc.vector.tensor_tensor(ng[:], pyr[:], ng[:], op=OP.max)
                nc.vector.tensor_scalar(ng[:], ng[:], -1.0, 1.0,
                                        op0=OP.mult, op1=OP.add)
                nc.vector.tensor_scalar(ty1[:], ng[:], 0.0, hmc,
                                        op0=OP.max, op1=OP.mult)
            return (ty0, ty1, ty2)

        s1(0)
        s1(1)
        tys_next = emit_ty(0)
        for r in range(RH):
            s1(r + 2)
            tys = tys_next

            # squares pre-scaled by sqrt(V) on ACT: Square(sqrt(V)*w) = V*w^2,
            # so the variance step is a plain 2x-mode subtract (no stt).
            sqscale = all(ch == "a" for ch in _SQ_ENG[:5])
            rtv = float(np.sqrt(V)) if sqscale else 1.0
            f0row = rw2_p.tile([128, CJ], F16, tag="f0row")
            _F0_DMA(nc).dma_start(
                f0row[:], bass.AP(f0s, r * (2 * CJ),
                                  [[CJ, 2], [0, 64], [1, CJ]]))
            vsq = row_p.tile([128, CJ], F16, tag="vsq")
            if _SQ_ENG[0] == "a":
                nc.scalar.activation(vsq[:], f0row[:], ACT.Square, scale=rtv)
            else:
                _eng(nc, _SQ_ENG[0]).tensor_tensor(vsq[:], f0row[:],
                                                   f0row[:], op=OP.mult)

            if _VS_ENG == "p":
                vs_ps = pv_p.tile([128, CJ], F32, tag="vsps")
                for ck in range(0, CJ, 512):
                    nc.tensor.matmul(vs_ps[:, ck:ck + 512], identf[:],
                                     f0row[:, ck:ck + 512],
                                     start=True, stop=False)
            else:
                vsum = row_p.tile([128, CJ], F16, tag="vsum")
            ea0 = _eng(nc, _ACC_ENG[0])
            ea1 = _eng(nc, _ACC_ENG[1])
            for vi in range(NVIEW):
                E = _eng(nc, _S2_ENG[vi])
                wv = s2_p.tile([128, CJ], F16, tag="warped")
                wvv = wv[:].rearrange("p (c x) -> p c x", x=W)
                for jj in range(3):
                    g = (gslot[(r + jj) % 3][:, vi * CJ:(vi + 1) * CJ]
                         .rearrange("p (c x) -> p c x", x=W))
                    t = (tys[jj][:, vi * W:(vi + 1) * W]
                         .unsqueeze(1).broadcast_to([128, C16, W]))
                    if jj == 0:
                        E.tensor_tensor(wvv, t, g, op=OP.mult)
                    else:
                        tw = s1_p.tile([128, CJ], F16, tag="tmpS1")
                        twv = tw[:].rearrange("p (c x) -> p c x", x=W)
                        E.tensor_tensor(twv, t, g, op=OP.mult)
                        E.tensor_tensor(wvv, wvv, twv, op=OP.add)
                if _VS_ENG == "p":
                    for ck in range(0, CJ, 512):
                        nc.tensor.matmul(vs_ps[:, ck:ck + 512], identf[:],
                                         wv[:, ck:ck + 512], start=False,
                                         stop=(vi == NVIEW - 1))
                elif vi == 0:
                    ea0.tensor_tensor(vsum[:], f0row[:], wv[:], op=OP.add)
                else:
                    ea0.tensor_tensor(vsum[:], vsum[:], wv[:], op=OP.add)
                sqv = rw2_p.tile([128, CJ], F16, tag="sqv")
                if _SQ_ENG[1 + vi] == "a":
                    nc.scalar.activation(sqv[:], wv[:], ACT.Square, scale=rtv)
                else:
                    _eng(nc, _SQ_ENG[1 + vi]).tensor_tensor(
                        sqv[:], wv[:], wv[:], op=OP.mult)
                ea1.tensor_tensor(vsq[:], vsq[:], sqv[:], op=OP.add)

            # variance scaled by V^2 (host folds 1/V^2 into the conv band):
            # varr = V*vsq - vsum^2
            m = m_p.tile([128, CJ], F16, tag="m")
            msrc = vs_ps if _VS_ENG == "p" else vsum
            if _SQ_ENG[5] == "a":
                nc.scalar.activation(m[:], msrc[:], ACT.Square)
            else:
                _eng(nc, _SQ_ENG[5]).tensor_tensor(m[:], msrc[:], msrc[:],
                                                   op=OP.mult)
            varr = (vslot[r % 3][:].rearrange("p (c x) -> p c x", x=WP)
                    [:, :, 1:W + 1])
            if sqscale:
                nc.vector.tensor_tensor(
                    varr, vsq[:].rearrange("p (c x) -> p c x", x=W),
                    m[:].rearrange("p (c x) -> p c x", x=W), op=OP.subtract)
            elif _VAR_ENG in ("2", "v"):
                tv = s1_p.tile([128, CJ], F16, tag="tmpS1")
                nc.vector.tensor_scalar(tv[:], vsq[:], float(V), None,
                                        op0=OP.mult)
                sube = nc.gpsimd if _VAR_ENG == "2" else nc.vector
                sube.tensor_tensor(
                    varr, tv[:].rearrange("p (c x) -> p c x", x=W),
                    m[:].rearrange("p (c x) -> p c x", x=W), op=OP.subtract)
            else:
                nc.vector.scalar_tensor_tensor(
                    varr, vsq[:].rearrange("p (c x) -> p c x", x=W),
                    float(V), m[:].rearrange("p (c x) -> p c x", x=W),
                    op0=OP.mult, op1=OP.subtract)
            # prefetch next row's tents (double-buffered) so S2(r+1) never
            # waits on the ACT tent chain at row start
            if r + 1 < RH:
                tys_next = emit_ty(r + 1)
            if r >= 2:
                conv_row(r - 1)


def _get_runner(nrep=1):
    key = (nrep, _GEOM, _S1_ENG, _S2_ENG, _ACC_ENG, _VAR_ENG, _TY_ENG,
           _VS_ENG, _SQ_ENG,
           tuple(os.environ.get(k, "") for k in
                 ("K_FRQ", "K_F0Q", "K_OUTQ", "K_FRSPLIT")))
    if key in _cache:
        return _cache[key]
    import jax
    from jax.sharding import Mesh, PartitionSpec
    from jax.experimental.shard_map import shard_map
    from concourse.bass2jax import (_bass_exec_p, install_neuronx_cc_hook,
                                    partition_id_tensor)

    nc = _build_program(nrep, _GEOM)
    install_neuronx_cc_hook()
    partition_name = (nc.partition_id_tensor.name
                      if nc.partition_id_tensor else None)
    in_names, out_names, out_avals, zero_outs = [], [], [], []
    for alloc in nc.m.functions[0].allocations:
        if not isinstance(alloc, mybir.MemoryLocationSet):
            continue
        name = alloc.memorylocations[0].name
        if alloc.kind == "ExternalInput":
            if name != partition_name:
                in_names.append(name)
        elif alloc.kind == "ExternalOutput":
            shape = tuple(alloc.tensor_shape)
            dtype = mybir.dt.np(alloc.dtype)
            out_names.append(name)
            out_avals.append(jax.core.ShapedArray(shape, dtype))
            zero_outs.append(np.zeros(shape, dtype))
    n_params, n_outs = len(in_names), len(out_avals)
    all_in = list(in_names) + list(out_names) + (
        [partition_name] if partition_name else [])

    def _body(*args):
        operands = list(args)
        if partition_name is not None:
            operands.append(partition_id_tensor())
        outs = _bass_exec_p.bind(
            *operands, out_avals=tuple(out_avals), in_names=tuple(all_in),
            out_names=tuple(out_names), lowering_input_output_aliases=(),
            sim_require_finite=True, sim_require_nnan=True, nc=nc)
        return tuple(outs)

    devices = jax.devices()[:NCORES]
    mesh = Mesh(np.asarray(devices), ("core",))
    in_specs = (PartitionSpec("core"),) * (n_params + n_outs)
    out_specs = (PartitionSpec("core"),) * n_outs
    sharded = jax.jit(
        shard_map(_body, mesh=mesh, in_specs=in_specs, out_specs=out_specs,
                  check_rep=False), keep_unused=True)

    from jax.sharding import NamedSharding
    shard = NamedSharding(mesh, PartitionSpec("core"))
    dev_cache = {}

    def run(in_maps, fetch=True):
        ck = id(in_maps)
        if ck not in dev_cache:
            per_core = [[np.asarray(m[n]) for n in in_names] for m in in_maps]
            concat_in = [
                np.concatenate([per_core[c][i] for c in range(NCORES)], axis=0)
                for i in range(n_params)]
            concat_zeros = [
                np.zeros((NCORES * z.shape[0], *z.shape[1:]), z.dtype)
                for z in zero_outs]
            dev_cache.clear()
            dev_cache[ck] = [jax.device_put(x, shard)
                             for x in concat_in + concat_zeros]
            jax.block_until_ready(dev_cache[ck])
        out_arrs = sharded(*dev_cache[ck])
        jax.block_until_ready(out_arrs)
        if not fetch:
            return None
        return [{n: np.asarray(out_arrs[i]).reshape(
                    NCORES, *out_avals[i].shape)[c]
                 for i, n in enumerate(out_names)} for c in range(NCORES)]

    _cache[key] = run
    return run


def _host_prep(feat0, feat1, feat2, feat3, feat4, proj_matrices, depth_values,
               conv_w):
    global _GEOM
    feats = [np.asarray(f, np.float32) for f in
             (feat0, feat1, feat2, feat3, feat4)]
    projs = np.asarray(proj_matrices, np.float64)
    depth = np.asarray(depth_values, np.float64)[0]          # [D]
    w3 = np.asarray(conv_w, np.float32)[0]                   # [C,3,3,3]

    def fuse(p):  # p [2,4,4]
        out = p[0].copy()
        out[:3, :4] = p[1, :3, :3] @ p[0, :3, :4]
        return out

    ref = fuse(projs[0, 0])
    ref_inv = np.linalg.inv(ref)
    Rs, ts = [], []
    for v in range(1, V):
        P = fuse(projs[0, v]) @ ref_inv
        Rs.append(P[:3, :3])
        ts.append(P[:3, 3])
        assert abs(P[0, 1]) < 1e-5 and abs(P[2, 1]) < 1e-5, "px depends on y"
        assert abs(P[1, 1] - 1.0) < 1e-5, "py y-slope != 1"

    # per-view window geometry (shared by all cores); p = (cl, d)
    dgrid = np.arange(128) % 64
    dep = depth[dgrid]                                       # [128]
    xg = np.arange(W, dtype=np.float64)[None, :]             # [1, W]

    sxs, ncols_l, pxs = [], [], []
    for v in range(1, V):
        R, t = Rs[v - 1], ts[v - 1]
        den = (R[2, 0] * xg + R[2, 2]) * dep[:, None] + t[2]
        px = ((R[0, 0] * xg + R[0, 2]) * dep[:, None] + t[0]) / den
        rel = px - xg
        sx = int(np.floor(rel.min()))
        nc_ = int(np.floor(rel.max())) + 2 - sx
        assert 2 <= nc_ <= 3, f"view {v}: ncols={nc_}"
        assert nc_ - 1 + W <= FW, "window fits"
        sxs.append(sx)
        ncols_l.append(nc_)
        pxs.append(px)

    # tent weights for x + active-x runs per (view, tap)
    txall = np.zeros((128, sum(ncols_l) * W), np.float16)
    runs_l = []
    off = 0
    for vi in range(NVIEW):
        fx = pxs[vi] - xg - sxs[vi]
        nc_ = ncols_l[vi]
        if nc_ == 2:
            assert fx.min() > 0 and fx.max() < 1
            tents = [1.0 - fx, fx]
        else:
            assert fx.min() > 0 and fx.max() < 2
            tents = [np.maximum(0.0, 1.0 - fx),
                     1.0 - np.abs(fx - 1.0),
                     np.maximum(0.0, fx - 1.0)]
        # order taps so a full-width tap comes first (it is the writer)
        vruns = []
        for i, tn in enumerate(tents):
            txall[:, off + i * W: off + (i + 1) * W] = tn.astype(np.float16)
            active = (tn > 0).any(axis=0)                    # [W]
            if active.all():
                vruns.append(((0, W),))
            else:
                # contiguous runs of active columns
                idx = np.flatnonzero(active)
                assert len(idx) > 0
                brk = np.flatnonzero(np.diff(idx) > 1)
                starts = np.concatenate([[idx[0]], idx[brk + 1]])
                ends = np.concatenate([idx[brk] + 1, [idx[-1] + 1]])
                vruns.append(tuple((int(a), int(b))
                                   for a, b in zip(starts, ends)))
        order = sorted(range(nc_), key=lambda i: vruns[i] != ((0, W),))
        assert vruns[order[0]] == ((0, W),), f"view {vi + 1}: no full tap"
        # reorder tents in txall to match emission order
        tx2 = txall[:, off:off + nc_ * W].copy()
        vr2 = []
        for pos, i in enumerate(order):
            txall[:, off + pos * W: off + (pos + 1) * W] = \
                tx2[:, i * W:(i + 1) * W]
            vr2.append(vruns[i])
        # emission reads fr columns [i + xa, i + xb): keep tap index mapping
        runs_l.append(tuple((order[pos], tuple(vr2[pos]))
                            for pos in range(nc_)))
        off += nc_ * W
    _GEOM = (tuple(ncols_l), tuple(runs_l))

    # staged window tensor: fwinall[Y+2, cl, v, c16*FW] (channel ch=2*c16+cl)
    fwin_full = np.zeros((H + 4, 2, NVIEW, C16, FW), np.float16)
    for v in range(1, V):
        fpad = np.zeros((H + 4, C, 256), np.float16)
        fpad[2:H + 2, :, :W] = feats[v][0].transpose(1, 0, 2)
        lo = sxs[v - 1]
        for cl in range(2):
            fwin_full[:, cl, v - 1] = fpad[:, cl::2, lo:lo + FW]

    # conv band matrices (shared), fp16, with 1/V^2 folded in
    band = np.zeros((9, NQ, 128, 64), np.float32)
    d_ = np.arange(64)
    dz = d_[:, None] - d_[None, :] + 1
    msk = (dz >= 0) & (dz < 3)
    dzc = np.clip(dz, 0, 2)
    for dy in range(3):
        for dx in range(3):
            for k in range(NQ):
                for cl in range(2):
                    c = 2 * k + cl
                    blk = np.where(msk, w3[c, dzc, dy, dx] / (V * V), 0.0)
                    band[dy * 3 + dx, k, cl * 64:(cl + 1) * 64, :] = blk
    band = band.reshape(9 * NQ, 128, 64).astype(np.float16)
    ident = np.eye(128, dtype=np.float16)

    f0pad = np.zeros((H + 2, C, W), np.float32)
    f0pad[1:H + 1] = feats[0][0].transpose(1, 0, 2)

    in_maps = []
    for core in range(NCORES):
        base = core * ROWS
        # y-interp coefficients: pyr = A'' + r*B1 per (p, view, x)
        aall = np.zeros((128, NVIEW * W), np.float32)
        ball = np.zeros((128, NVIEW * W), np.float32)
        for vi in range(NVIEW):
            R, t = Rs[vi], ts[vi]
            den = (R[2, 0] * xg + R[2, 2]) * dep[:, None] + t[2]
            rd = 1.0 / den
            ny0 = (R[1, 0] * xg + R[1, 2] + (base - 1)) * dep[:, None] + t[1]
            a2 = ny0 * rd - (base - 1)
            b1 = dep[:, None] * rd - 1.0
            aall[:, vi * W:(vi + 1) * W] = a2
            ball[:, vi * W:(vi + 1) * W] = b1
        # halo masks: rows outside [0, H)
        hmask = np.zeros((128, 2 * RH), np.float32)
        for r in range(RH):
            y = base - 1 + r
            mval = 1.0 if 0 <= y < H else 0.0
            hmask[:, r] = mval
            hmask[:, RH + r] = -mval
        # ref feature slab rows base-1 .. base+16, channel-parity split
        f0slab = np.zeros((RH, 2, C16, W), np.float16)
        for r in range(RH):
            for cl in range(2):
                f0slab[r, cl] = (f0pad[base + r, cl::2, :]
                                 .astype(np.float16))
        m = dict(fwinall=fwin_full[base:base + SRC]
                 .reshape(SRC, 2, NVIEW * CFW).copy(),
                 f0slab=f0slab.reshape(RH, 2, CJ), a_all=aall, b1_all=ball,
                 txall=txall, hmask=hmask, band=band, ident=ident)
        in_maps.append(m)
    return in_maps


def kernel(feat0, feat1, feat2, feat3, feat4, proj_matrices, depth_values,
           num_depth=None, conv_w=None, conv_b=None, **_):
    in_maps = _host_prep(feat0, feat1, feat2, feat3, feat4, proj_matrices,
                         depth_values, conv_w)
    run = _get_runner(1)
    res = run(in_maps)
    out = np.zeros((B, D, H, W), np.float32)
    for core in range(NCORES):
        o = res[core]["out"]                                 # [ROWS, D, W]
        out[0, :, core * ROWS:(core + 1) * ROWS, :] = o.transpose(1, 0, 2)
    return out


# revision 51
# speedup vs baseline: 1.0098x; 1.0098x over previous
"""DepthNet cost-volume kernel for 8 Trainium2 NeuronCores (v3, cl-layout).

Partition layout: p = (cl, d) where cl = channel parity (2) and d = depth
(64); free dims are (c16, x) with c16 = channel-within-parity (16) and
x = full row width (160).  Channel ch = 2*c16 + cl.

Why: the 3x3x3 conv contracts (channel-pair, depth-band) on the PE with a
banded lhsT -- in this layout the variance ring rows ARE the conv rhs
(x-contiguous per (c16, slot)), so the former SBUF->SBUF relayout DMAs
(4 per row) disappear entirely.  x fully in the free dim also lets the
3rd bilinear column tap run only over the x-runs where the x-offset
straddles an integer (host-computed), instead of full width.

Per row r (16 output rows + 1 conv-halo row each side per core):
  S1 (x-interp, shared by 3 output rows): per (view, source row Y)
     Gx[c,x,d] = sum_i tentx_i(px(x,d)) * F[c,Y,x+sx+i]
     features replicated across the 64 d-partitions by one stride-0
     HWDGE DMA from a host-staged window tensor (all views in one DMA).
  S2 (y-interp): warped = sum_j tenty_j(py(x,y,d)) * Gx[y-1+j]
     vsum accumulated on PE (identity matmuls into PSUM), squares +
     vsq adds per engine knobs, variance = V*vsq - vsum^2 into an SBUF
     ring (1/V^2 folded into the conv band host-side).
  Conv (interleaved, one row behind): 3x3x3 conv = 144 PSUM-accumulated
     banded matmuls reading the variance ring directly; then softmax
     over depth and transposed store.
No cross-core communication.  Wide elementwise ops run fp16 (2x DVE).
"""
import os
import numpy as np
import concourse.bass as bass
import concourse.tile as tile
from concourse import bacc, mybir

F32 = mybir.dt.float32
F16 = mybir.dt.float16
OP = mybir.AluOpType
ACT = mybir.ActivationFunctionType

B, C, H, W, D, V = 1, 32, 128, 160, 64, 5
NCORES = 8
ROWS = H // NCORES          # output rows per core
RH = ROWS + 2               # with conv halo
SRC = RH + 2                # source rows touched per core
NQ = 16                     # conv channel-pair chunks
C16 = C // 2                # channels per parity
FW = 162                    # staged window width per view
CJ = C16 * W                # free size of one view-row block (2560)
CFW = C16 * FW
NVIEW = V - 1
WP = W + 2                  # padded variance width

_cache = {}
_GEOM = None                # (ncols tuple, runs) set by _host_prep

# engine split (v=DVE, g=Pool), tunable via env
_S1_ENG = os.environ.get("K_S1", "vvvv")   # per-view S1 x-interp
_S2_ENG = os.environ.get("K_S2", "vvvv")   # per-view S2 y-interp
_ACC_ENG = os.environ.get("K_ACC", "vv")   # vsum, vsq adds
_VAR_ENG = os.environ.get("K_VAR", "s")    # variance: 2=split, s=stt DVE
_TY_ENG = os.environ.get("K_TY", "a")      # tenty smalls: a=ACT, v=DVE
_VS_ENG = os.environ.get("K_VS", "p")      # vsum: p=PE psum, v/g=tt adds
_SQ_ENG = os.environ.get("K_SQ", "aaaaaa")  # squares: f0,4 views,m; a/v/g


def _build_program(nrep, geom):
    ncols, runs = geom
    nc = bacc.Bacc("TRN2", target_bir_lowering=False, debug=False,
                   num_devices=NCORES, num_swdge_queues=4)
    fwinall = nc.dram_tensor("fwinall", [SRC, 2, NVIEW * CFW], F16,
                             kind="ExternalInput")
    f0s = nc.dram_tensor("f0slab", [RH, 2, CJ], F16, kind="ExternalInput")
    a_all = nc.dram_tensor("a_all", [128, NVIEW * W], F32,
                           kind="ExternalInput")
    b1_all = nc.dram_tensor("b1_all", [128, NVIEW * W], F32,
                            kind="ExternalInput")
    txh = nc.dram_tensor("txall", [128, sum(ncols) * W], F16,
                         kind="ExternalInput")
    hmaskh = nc.dram_tensor("hmask", [128, 2 * RH], F32, kind="ExternalInput")
    bandh = nc.dram_tensor("band", [9 * NQ, 128, 64], F16,
                           kind="ExternalInput")
    identh = nc.dram_tensor("ident", [128, 128], F16, kind="ExternalInput")
    outh = nc.dram_tensor("out", [ROWS, D, W], F32, kind="ExternalOutput")

    with tile.TileContext(nc) as tc:
        import contextlib
        with contextlib.ExitStack() as ctx:
            const_p = ctx.enter_context(tc.tile_pool(name="const", bufs=1))
            aA = const_p.tile([128, NVIEW * W], F32)
            nc.sync.dma_start(aA[:], a_all.ap())
            bB = const_p.tile([128, NVIEW * W], F32)
            nc.sync.dma_start(bB[:], b1_all.ap())
            tx = const_p.tile([128, sum(ncols) * W], F16)
            nc.sync.dma_start(tx[:], txh.ap())
            hm = const_p.tile([128, 2 * RH], F32)
            nc.sync.dma_start(hm[:], hmaskh.ap())
            band = const_p.tile([128, 9 * NQ * 64], F16)
            nc.sync.dma_start(
                band[:],
                bass.AP(bandh, 0, [[64, 128], [8192, 9 * NQ], [1, 64]]))
            ident = const_p.tile([128, 128], F16)
            nc.sync.dma_start(ident[:], identh.ap())

            for rep in range(nrep):
                if rep > 0:
                    tc.strict_bb_all_engine_barrier()
                _emit_main(tc, nc, ncols, runs, fwinall, f0s, aA, bB, tx, hm,
                           band, ident, outh)
    nc.compile()
    return nc


def _eng(nc, ch):
    return nc.vector if ch == "v" else nc.gpsimd


_DMAQ_MAP = {"a": "scalar", "s": "sync", "p": "gpsimd"}


def _dmaq(nc, ch):
    return getattr(nc, _DMAQ_MAP[ch])


def _FR_DMA(nc):
    return _dmaq(nc, os.environ.get("K_FRQ", "s")[0])


def _F0_DMA(nc):
    return _dmaq(nc, os.environ.get("K_F0Q", "s")[0])


def _OUT_DMA(nc):
    return _dmaq(nc, os.environ.get("K_OUTQ", "s")[0])


def _emit_main(tc, nc, ncols, runs, fwinall, f0s, aA, bB, tx, hm, band,
               ident, outh):
    import contextlib
    txoff = [sum(ncols[:i]) * W for i in range(NVIEW)]
    with contextlib.ExitStack() as st:
        gx_p = st.enter_context(tc.tile_pool(name="gx", bufs=1))
        frep_p = st.enter_context(tc.tile_pool(name="frep", bufs=2))
        # tmpS1 is written+consumed back-to-back on the (in-order) DVE, so a
        # single buffer loses no overlap and frees 5KB/partition
        s1_p = st.enter_context(tc.tile_pool(name="s1", bufs=1))
        s2_p = st.enter_context(tc.tile_pool(name="s2", bufs=2))
        row_p = st.enter_context(tc.tile_pool(name="row", bufs=2))
        rw2_p = st.enter_context(tc.tile_pool(name="rw2", bufs=2))
        ty_p = st.enter_context(tc.tile_pool(name="ty", bufs=1))
        typ2 = st.enter_context(tc.tile_pool(name="typ2", bufs=2))
        m_p = st.enter_context(tc.tile_pool(name="msq", bufs=1))
        vr_p = st.enter_context(tc.tile_pool(name="vr", bufs=1))
        pp = st.enter_context(tc.tile_pool(name="cpsum", bufs=1, space="PSUM"))
        sp = st.enter_context(tc.tile_pool(name="soft", bufs=2))

        gslot, vslot = [], []
        for s in range(3):
            gs = gx_p.tile([128, NVIEW * CJ], F16, tag=f"gs{s}")
            gslot.append(gs)
            vs = vr_p.tile([128, C16 * WP], F16, tag=f"vr{s}")
            # zero the x-pad columns once; rows only ever write [1:W+1]
            nc.vector.memset(
                vs[:].rearrange("p (c x) -> p c x", x=WP)[:, :, 0:WP:WP - 1],
                0.0)
            vslot.append(vs)

        pv_p = st.enter_context(tc.tile_pool(name="vpsum", bufs=1,
                                             space="PSUM"))
        identf = ident
        ones64 = gx_p.tile([64, 64], F16, tag="ones64")
        nc.vector.memset(ones64[:], 1.0)

        def s1(ssi):
            # x-interp source row ssi (global Y = base-2+ssi) for all views
            s = ssi % 3
            fr = frep_p.tile([128, NVIEW * CFW], F16, tag="frall")
            nsp = int(os.environ.get("K_FRSPLIT", "1"))
            seg = NVIEW * CFW // nsp
            for si in range(nsp):
                in_ap = bass.AP(fwinall, ssi * (NVIEW * 2 * CFW) + si * seg,
                                [[NVIEW * CFW, 2], [0, 64], [1, seg]])
                _FR_DMA(nc).dma_start(fr[:, si * seg:(si + 1) * seg], in_ap)
            frv = fr[:].rearrange("p (v c w) -> p v c w", c=C16, w=FW)
            for vi in range(NVIEW):
                gsub = (gslot[s][:, vi * CJ:(vi + 1) * CJ]
                        .rearrange("p (c x) -> p c x", x=W))
                E = _eng(nc, _S1_ENG[vi])
                first = True
                for pos in range(ncols[vi]):
                    orig_i, rlist = runs[vi][pos]
                    for (xa, xb) in rlist:
                        txv = (tx[:, txoff[vi] + pos * W + xa:
                                  txoff[vi] + pos * W + xb]
                               .unsqueeze(1).broadcast_to([128, C16, xb - xa]))
                        fseg = frv[:, vi, :, orig_i + xa:orig_i + xb]
                        if first:
                            assert (xa, xb) == (0, W), \
                                "first S1 tap must cover full width"
                            E.tensor_tensor(gsub, txv, fseg, op=OP.mult)
                            first = False
                        else:
                            tm = s1_p.tile([128, CJ], F16, tag="tmpS1")
                            tmv = (tm[:].rearrange("p (c x) -> p c x", x=W)
                                   [:, :, xa:xb])
                            E.tensor_tensor(tmv, txv, fseg, op=OP.mult)
                            E.tensor_tensor(gsub[:, :, xa:xb],
                                            gsub[:, :, xa:xb], tmv, op=OP.add)

        pair = {}

        def conv_row(ro):
            cost = pp.tile([64, W], F32, tag="cost")
            first = True
            for dy in range(3):
                vsv = (vslot[(ro + dy - 1) % 3][:]
                       .rearrange("p (c x) -> p c x", x=WP))
                for dx in range(3):
                    t = dy * 3 + dx
                    for k in range(NQ):
                        rhs = vsv[:, k, dx:dx + W]
                        lhsT = band[:, (t * NQ + k) * 64:(t * NQ + k + 1) * 64]
                        last = (dy == 2 and dx == 2 and k == NQ - 1)
                        nc.tensor.matmul(cost[:], lhsT, rhs,
                                         start=first, stop=last)
                        first = False
            # transpose-free softmax over d (on partitions): |cost| is small
            # (conv of variance with ~0.05-scale weights), so exp needs no
            # max subtraction; depth-sum via all-ones PE matmul replicates
            # the denominator across all 64 partitions.  The tail (sum,
            # reciprocal, scale, store) is batched over row pairs to halve
            # the instruction and DMA-issue count.
            if not pair:
                e2 = sp.tile([64, 2 * W], F16, tag="e")
                pair["e"] = e2
                pair["ro"] = ro
                nc.scalar.activation(e2[:, 0:W], cost[:], ACT.Exp)
                return
            e2 = pair.pop("e")
            ro0 = pair.pop("ro")
            assert ro == ro0 + 1
            nc.scalar.activation(e2[:, W:2 * W], cost[:], ACT.Exp)
            ssum = pp.tile([64, 2 * W], F32, tag="ssum")
            nc.tensor.matmul(ssum[:], ones64[:], e2[:], start=True, stop=True)
            rinv = sp.tile([64, 2 * W], F16, tag="rinv")
            with nc.allow_low_precision(reason="softmax denom fp16 is ample"):
                nc.vector.reciprocal(rinv[:], ssum[:])
            prob = m_p.tile([64, 2 * W], F32, tag="prob")
            nc.vector.tensor_tensor(prob[:], e2[:], rinv[:], op=OP.mult)
            out_ap = bass.AP(outh, (ro0 - 1) * D * W,
                             [[W, 64], [D * W, 2], [1, W]])
            _OUT_DMA(nc).dma_start(out_ap, prob[:])

        def emit_ty(r):
            # tenty weights for the 3 source rows (masked for halo rows)
            pyr = ty_p.tile([128, NVIEW * W], F32, tag="pyr")
            nc.vector.scalar_tensor_tensor(pyr[:], bB[:], float(r), aA[:],
                                           op0=OP.mult, op1=OP.add)
            hmc = hm[:, r:r + 1]
            hmn = hm[:, RH + r:RH + r + 1]
            ty0 = typ2.tile([128, NVIEW * W], F16, tag="ty0")
            ty1 = typ2.tile([128, NVIEW * W], F16, tag="ty1")
            ty2 = typ2.tile([128, NVIEW * W], F16, tag="ty2")
            if _TY_ENG == "a":
                # masked tents on ACT: Relu(scale*x + bias), per-part scale
                nc.scalar.activation(ty0[:], pyr[:], ACT.Relu, scale=hmn)
                nc.scalar.activation(ty2[:], pyr[:], ACT.Relu, scale=hmc)
                ab = ty_p.tile([128, NVIEW * W], F16, tag="ng")
                nc.scalar.activation(ab[:], pyr[:], ACT.Abs)
                nc.scalar.activation(ty1[:], ab[:], ACT.Relu,
                                     scale=hmn, bias=hmc)
            else:
                nc.vector.tensor_scalar(ty0[:], pyr[:], hmn, 0.0,
                                        op0=OP.mult, op1=OP.max)
                nc.vector.tensor_scalar(ty2[:], pyr[:], hmc, 0.0,
                                        op0=OP.mult, op1=OP.max)
                ng = ty_p.tile([128, NVIEW * W], F32, tag="ng")
                nc.vector.tensor_scalar(ng[:], pyr[:], -1.0, None,
                                        op0=OP.mult)
                nc.vector.tensor_tensor(ng[:], pyr[:], ng[:], op=OP.max)
                nc.vector.tensor_scalar(ng[:], ng[:], -1.0, 1.0,
                                        op0=OP.mult, op1=OP.add)
                nc.vector.tensor_scalar(ty1[:], ng[:], 0.0, hmc,
                                        op0=OP.max, op1=OP.mult)
            return (ty0, ty1, ty2)

        s1(0)
        s1(1)
        for r in range(RH):
            s1(r + 2)
            tys = emit_ty(r)

            # squares pre-scaled by sqrt(V) on ACT: Square(sqrt(V)*w) = V*w^2,
            # so the variance step is a plain 2x-mode subtract (no stt).
            sqscale = all(ch == "a" for ch in _SQ_ENG[:5])
            rtv = float(np.sqrt(V)) if sqscale else 1.0
            f0row = rw2_p.tile([128, CJ], F16, tag="f0row")
            _F0_DMA(nc).dma_start(
                f0row[:], bass.AP(f0s, r * (2 * CJ),
                                  [[CJ, 2], [0, 64], [1, CJ]]))
            vsq = row_p.tile([128, CJ], F16, tag="vsq")
            if _SQ_ENG[0] == "a":
                nc.scalar.activation(vsq[:], f0row[:], ACT.Square, scale=rtv)
            else:
                _eng(nc, _SQ_ENG[0]).tensor_tensor(vsq[:], f0row[:],
                                                   f0row[:], op=OP.mult)

            if _VS_ENG == "p":
                vs_ps = pv_p.tile([128, CJ], F32, tag="vsps")
                for ck in range(0, CJ, 512):
                    nc.tensor.matmul(vs_ps[:, ck:ck + 512], identf[:],
                                     f0row[:, ck:ck + 512],
                                     start=True, stop=False)
            else:
                vsum = row_p.tile([128, CJ], F16, tag="vsum")
            ea0 = _eng(nc, _ACC_ENG[0])
            ea1 = _eng(nc, _ACC_ENG[1])
            for vi in range(NVIEW):
                E = _eng(nc, _S2_ENG[vi])
                wv = s2_p.tile([128, CJ], F16, tag="warped")
                wvv = wv[:].rearrange("p (c x) -> p c x", x=W)
                for jj in range(3):
                    g = (gslot[(r + jj) % 3][:, vi * CJ:(vi + 1) * CJ]
                         .rearrange("p (c x) -> p c x", x=W))
                    t = (tys[jj][:, vi * W:(vi + 1) * W]
                         .unsqueeze(1).broadcast_to([128, C16, W]))
                    if jj == 0:
                        E.tensor_tensor(wvv, t, g, op=OP.mult)
                    else:
                        tw = s1_p.tile([128, CJ], F16, tag="tmpS1")
                        twv = tw[:].rearrange("p (c x) -> p c x", x=W)
                        E.tensor_tensor(twv, t, g, op=OP.mult)
                        E.tensor_tensor(wvv, wvv, twv, op=OP.add)
                if _VS_ENG == "p":
                    for ck in range(0, CJ, 512):
                        nc.tensor.matmul(vs_ps[:, ck:ck + 512], identf[:],
                                         wv[:, ck:ck + 512], start=False,
                                         stop=(vi == NVIEW - 1))
                elif vi == 0:
                    ea0.tensor_tensor(vsum[:], f0row[:], wv[:], op=OP.add)
                else:
                    ea0.tensor_tensor(vsum[:], vsum[:], wv[:], op=OP.add)
                sqv = rw2_p.tile([128, CJ], F16, tag="sqv")
                if _SQ_ENG[1 + vi] == "a":
                    nc.scalar.activation(sqv[:], wv[:], ACT.Square, scale=rtv)
                else:
                    _eng(nc, _SQ_ENG[1 + vi]).tensor_tensor(
                        sqv[:], wv[:], wv[:], op=OP.mult)
                ea1.tensor_tensor(vsq[:], vsq[:], sqv[:], op=OP.add)

            # variance scaled by V^2 (host folds 1/V^2 into the conv band):
            # varr = V*vsq - vsum^2
            m = m_p.tile([128, CJ], F16, tag="m")
            msrc = vs_ps if _VS_ENG == "p" else vsum
            if _SQ_ENG[5] == "a":
                nc.scalar.activation(m[:], msrc[:], ACT.Square)
            else:
                _eng(nc, _SQ_ENG[5]).tensor_tensor(m[:], msrc[:], msrc[:],
                                                   op=OP.mult)
            varr = (vslot[r % 3][:].rearrange("p (c x) -> p c x", x=WP)
                    [:, :, 1:W + 1])
            if sqscale:
                nc.vector.tensor_tensor(
                    varr, vsq[:].rearrange("p (c x) -> p c x", x=W),
                    m[:].rearrange("p (c x) -> p c x", x=W), op=OP.subtract)
            elif _VAR_ENG in ("2", "v"):
                tv = s1_p.tile([128, CJ], F16, tag="tmpS1")
                nc.vector.tensor_scalar(tv[:], vsq[:], float(V), None,
                                        op0=OP.mult)
                sube = nc.gpsimd if _VAR_ENG == "2" else nc.vector
                sube.tensor_tensor(
                    varr, tv[:].rearrange("p (c x) -> p c x", x=W),
                    m[:].rearrange("p (c x) -> p c x", x=W), op=OP.subtract)
            else:
                nc.vector.scalar_tensor_tensor(
                    varr, vsq[:].rearrange("p (c x) -> p c x", x=W),
                    float(V), m[:].rearrange("p (c x) -> p c x", x=W),
                    op0=OP.mult, op1=OP.subtract)
            if r >= 2:
                conv_row(r - 1)


def _get_runner(nrep=1):
    key = (nrep, _GEOM, _S1_ENG, _S2_ENG, _ACC_ENG, _VAR_ENG, _TY_ENG,
           _VS_ENG, _SQ_ENG,
           tuple(os.environ.get(k, "") for k in
                 ("K_FRQ", "K_F0Q", "K_OUTQ", "K_FRSPLIT")))
    if key in _cache:
        return _cache[key]
    import jax
    from jax.sharding import Mesh, PartitionSpec
    from jax.experimental.shard_map import shard_map
    from concourse.bass2jax import (_bass_exec_p, install_neuronx_cc_hook,
                                    partition_id_tensor)

    nc = _build_program(nrep, _GEOM)
    install_neuronx_cc_hook()
    partition_name = (nc.partition_id_tensor.name
                      if nc.partition_id_tensor else None)
    in_names, out_names, out_avals, zero_outs = [], [], [], []
    for alloc in nc.m.functions[0].allocations:
        if not isinstance(alloc, mybir.MemoryLocationSet):
            continue
        name = alloc.memorylocations[0].name
        if alloc.kind == "ExternalInput":
            if name != partition_name:
                in_names.append(name)
        elif alloc.kind == "ExternalOutput":
            shape = tuple(alloc.tensor_shape)
            dtype = mybir.dt.np(alloc.dtype)
            out_names.append(name)
            out_avals.append(jax.core.ShapedArray(shape, dtype))
            zero_outs.append(np.zeros(shape, dtype))
    n_params, n_outs = len(in_names), len(out_avals)
    all_in = list(in_names) + list(out_names) + (
        [partition_name] if partition_name else [])

    def _body(*args):
        operands = list(args)
        if partition_name is not None:
            operands.append(partition_id_tensor())
        outs = _bass_exec_p.bind(
            *operands, out_avals=tuple(out_avals), in_names=tuple(all_in),
            out_names=tuple(out_names), lowering_input_output_aliases=(),
            sim_require_finite=True, sim_require_nnan=True, nc=nc)
        return tuple(outs)

    devices = jax.devices()[:NCORES]
    mesh = Mesh(np.asarray(devices), ("core",))
    in_specs = (PartitionSpec("core"),) * (n_params + n_outs)
    out_specs = (PartitionSpec("core"),) * n_outs
    sharded = jax.jit(
        shard_map(_body, mesh=mesh, in_specs=in_specs, out_specs=out_specs,
                  check_rep=False), keep_unused=True)

    from jax.sharding import NamedSharding
    shard = NamedSharding(mesh, PartitionSpec("core"))
    dev_cache = {}

    def run(in_maps, fetch=True):
        ck = id(in_maps)
        if ck not in dev_cache:
            per_core = [[np.asarray(m[n]) for n in in_names] for m in in_maps]
            concat_in = [
                np.concatenate([per_core[c][i] for c in range(NCORES)], axis=0)
                for i in range(n_params)]
            concat_zeros = [
                np.zeros((NCORES * z.shape[0], *z.shape[1:]), z.dtype)
                for z in zero_outs]
            dev_cache.clear()
            dev_cache[ck] = [jax.device_put(x, shard)
                             for x in concat_in + concat_zeros]
            jax.block_until_ready(dev_cache[ck])
        out_arrs = sharded(*dev_cache[ck])
        jax.block_until_ready(out_arrs)
        if not fetch:
            return None
        return [{n: np.asarray(out_arrs[i]).reshape(
                    NCORES, *out_avals[i].shape)[c]
                 for i, n in enumerate(out_names)} for c in range(NCORES)]

    _cache[key] = run
    return run


def _host_prep(feat0, feat1, feat2, feat3, feat4, proj_matrices, depth_values,
               conv_w):
    global _GEOM
    feats = [np.asarray(f, np.float32) for f in
             (feat0, feat1, feat2, feat3, feat4)]
    projs = np.asarray(proj_matrices, np.float64)
    depth = np.asarray(depth_values, np.float64)[0]          # [D]
    w3 = np.asarray(conv_w, np.float32)[0]                   # [C,3,3,3]

    def fuse(p):  # p [2,4,4]
        out = p[0].copy()
        out[:3, :4] = p[1, :3, :3] @ p[0, :3, :4]
        return out

    ref = fuse(projs[0, 0])
    ref_inv = np.linalg.inv(ref)
    Rs, ts = [], []
    for v in range(1, V):
        P = fuse(projs[0, v]) @ ref_inv
        Rs.append(P[:3, :3])
        ts.append(P[:3, 3])
        assert abs(P[0, 1]) < 1e-5 and abs(P[2, 1]) < 1e-5, "px depends on y"
        assert abs(P[1, 1] - 1.0) < 1e-5, "py y-slope != 1"

    # per-view window geometry (shared by all cores); p = (cl, d)
    dgrid = np.arange(128) % 64
    dep = depth[dgrid]                                       # [128]
    xg = np.arange(W, dtype=np.float64)[None, :]             # [1, W]

    sxs, ncols_l, pxs = [], [], []
    for v in range(1, V):
        R, t = Rs[v - 1], ts[v - 1]
        den = (R[2, 0] * xg + R[2, 2]) * dep[:, None] + t[2]
        px = ((R[0, 0] * xg + R[0, 2]) * dep[:, None] + t[0]) / den
        rel = px - xg
        sx = int(np.floor(rel.min()))
        nc_ = int(np.floor(rel.max())) + 2 - sx
        assert 2 <= nc_ <= 3, f"view {v}: ncols={nc_}"
        assert nc_ - 1 + W <= FW, "window fits"
        sxs.append(sx)
        ncols_l.append(nc_)
        pxs.append(px)

    # tent weights for x + active-x runs per (view, tap)
    txall = np.zeros((128, sum(ncols_l) * W), np.float16)
    runs_l = []
    off = 0
    for vi in range(NVIEW):
        fx = pxs[vi] - xg - sxs[vi]
        nc_ = ncols_l[vi]
        if nc_ == 2:
            assert fx.min() > 0 and fx.max() < 1
            tents = [1.0 - fx, fx]
        else:
            assert fx.min() > 0 and fx.max() < 2
            tents = [np.maximum(0.0, 1.0 - fx),
                     1.0 - np.abs(fx - 1.0),
                     np.maximum(0.0, fx - 1.0)]
        # order taps so a full-width tap comes first (it is the writer)
        vruns = []
        for i, tn in enumerate(tents):
            txall[:, off + i * W: off + (i + 1) * W] = tn.astype(np.float16)
            active = (tn > 0).any(axis=0)                    # [W]
            if active.all():
                vruns.append(((0, W),))
            else:
                # contiguous runs of active columns
                idx = np.flatnonzero(active)
                assert len(idx) > 0
                brk = np.flatnonzero(np.diff(idx) > 1)
                starts = np.concatenate([[idx[0]], idx[brk + 1]])
                ends = np.concatenate([idx[brk] + 1, [idx[-1] + 1]])
                vruns.append(tuple((int(a), int(b))
                                   for a, b in zip(starts, ends)))
        order = sorted(range(nc_), key=lambda i: vruns[i] != ((0, W),))
        assert vruns[order[0]] == ((0, W),), f"view {vi + 1}: no full tap"
        # reorder tents in txall to match emission order
        tx2 = txall[:, off:off + nc_ * W].copy()
        vr2 = []
        for pos, i in enumerate(order):
            txall[:, off + pos * W: off + (pos + 1) * W] = \
                tx2[:, i * W:(i + 1) * W]
            vr2.append(vruns[i])
        # emission reads fr columns [i + xa, i + xb): keep tap index mapping
        runs_l.append(tuple((order[pos], tuple(vr2[pos]))
                            for pos in range(nc_)))
        off += nc_ * W
    _GEOM = (tuple(ncols_l), tuple(runs_l))

    # staged window tensor: fwinall[Y+2, cl, v, c16*FW] (channel ch=2*c16+cl)
    fwin_full = np.zeros((H + 4, 2, NVIEW, C16, FW), np.float16)
    for v in range(1, V):
        fpad = np.zeros((H + 4, C, 256), np.float16)
        fpad[2:H + 2, :, :W] = feats[v][0].transpose(1, 0, 2)
        lo = sxs[v - 1]
        for cl in range(2):
            fwin_full[:, cl, v - 1] = fpad[:, cl::2, lo:lo + FW]

    # conv band matrices (shared), fp16, with 1/V^2 folded in
    band = np.zeros((9, NQ, 128, 64), np.float32)
    d_ = np.arange(64)
    dz = d_[:, None] - d_[None, :] + 1
    msk = (dz >= 0) & (dz < 3)
    dzc = np.clip(dz, 0, 2)
    for dy in range(3):
        for dx in range(3):
            for k in range(NQ):
                for cl in range(2):
                    c = 2 * k + cl
                    blk = np.where(msk, w3[c, dzc, dy, dx] / (V * V), 0.0)
                    band[dy * 3 + dx, k, cl * 64:(cl + 1) * 64, :] = blk
    band = band.reshape(9 * NQ, 128, 64).astype(np.float16)
    ident = np.eye(128, dtype=np.float16)

    f0pad = np.zeros((H + 2, C, W), np.float32)
    f0pad[1:H + 1] = feats[0][0].transpose(1, 0, 2)

    in_maps = []
    for core in range(NCORES):
        base = core * ROWS
        # y-interp coefficients: pyr = A'' + r*B1 per (p, view, x)
        aall = np.zeros((128, NVIEW * W), np.float32)
        ball = np.zeros((128, NVIEW * W), np.float32)
        for vi in range(NVIEW):
            R, t = Rs[vi], ts[vi]
            den = (R[2, 0] * xg + R[2, 2]) * dep[:, None] + t[2]
            rd = 1.0 / den
            ny0 = (R[1, 0] * xg + R[1, 2] + (base - 1)) * dep[:, None] + t[1]
            a2 = ny0 * rd - (base - 1)
            b1 = dep[:, None] * rd - 1.0
            aall[:, vi * W:(vi + 1) * W] = a2
            ball[:, vi * W:(vi + 1) * W] = b1
        # halo masks: rows outside [0, H)
        hmask = np.zeros((128, 2 * RH), np.float32)
        for r in range(RH):
            y = base - 1 + r
            mval = 1.0 if 0 <= y < H else 0.0
            hmask[:, r] = mval
            hmask[:, RH + r] = -mval
        # ref feature slab rows base-1 .. base+16, channel-parity split
        f0slab = np.zeros((RH, 2, C16, W), np.float16)
        for r in range(RH):
            for cl in range(2):
                f0slab[r, cl] = (f0pad[base + r, cl::2, :]
                                 .astype(np.float16))
        m = dict(fwinall=fwin_full[base:base + SRC]
                 .reshape(SRC, 2, NVIEW * CFW).copy(),
                 f0slab=f0slab.reshape(RH, 2, CJ), a_all=aall, b1_all=ball,
                 txall=txall, hmask=hmask, band=band, ident=ident)
        in_maps.append(m)
    return in_maps


def kernel(feat0, feat1, feat2, feat3, feat4, proj_matrices, depth_values,
           num_depth=None, conv_w=None, conv_b=None, **_):
    in_maps = _host_prep(feat0, feat1, feat2, feat3, feat4, proj_matrices,
                         depth_values, conv_w)
    run = _get_runner(1)
    res = run(in_maps)
    out = np.zeros((B, D, H, W), np.float32)
    for core in range(NCORES):
        o = res[core]["out"]                                 # [ROWS, D, W]
        out[0, :, core * ROWS:(core + 1) * ROWS, :] = o.transpose(1, 0, 2)
    return out


# revision 52
# speedup vs baseline: 1.1683x; 1.1570x over previous
"""DepthNet cost-volume kernel for 8 Trainium2 NeuronCores (v3, cl-layout).

Partition layout: p = (cl, d) where cl = channel parity (2) and d = depth
(64); free dims are (c16, x) with c16 = channel-within-parity (16) and
x = full row width (160).  Channel ch = 2*c16 + cl.

Why: the 3x3x3 conv contracts (channel-pair, depth-band) on the PE with a
banded lhsT -- in this layout the variance ring rows ARE the conv rhs
(x-contiguous per (c16, slot)), so the former SBUF->SBUF relayout DMAs
(4 per row) disappear entirely.  x fully in the free dim also lets the
3rd bilinear column tap run only over the x-runs where the x-offset
straddles an integer (host-computed), instead of full width.

Per row r (16 output rows + 1 conv-halo row each side per core):
  S1 (x-interp, shared by 3 output rows): per (view, source row Y)
     Gx[c,x,d] = sum_i tentx_i(px(x,d)) * F[c,Y,x+sx+i]
     features replicated across the 64 d-partitions by one stride-0
     HWDGE DMA from a host-staged window tensor (all views in one DMA).
  S2 (y-interp): warped = sum_j tenty_j(py(x,y,d)) * Gx[y-1+j]
     vsum accumulated on PE (identity matmuls into PSUM), squares +
     vsq adds per engine knobs, variance = V*vsq - vsum^2 into an SBUF
     ring (1/V^2 folded into the conv band host-side).
  Conv (interleaved, one row behind): 3x3x3 conv = 144 PSUM-accumulated
     banded matmuls reading the variance ring directly; then softmax
     over depth and transposed store.
No cross-core communication.  Wide elementwise ops run fp16 (2x DVE).
"""
import os
import numpy as np
import concourse.bass as bass
import concourse.tile as tile
from concourse import bacc, mybir

F32 = mybir.dt.float32
F16 = mybir.dt.float16
OP = mybir.AluOpType
ACT = mybir.ActivationFunctionType

B, C, H, W, D, V = 1, 32, 128, 160, 64, 5
NCORES = 8
ROWS = H // NCORES          # output rows per core
RH = ROWS + 2               # with conv halo
SRC = RH + 2                # source rows touched per core
NQ = 16                     # conv channel-pair chunks
C16 = C // 2                # channels per parity
FW = 162                    # staged window width per view
CJ = C16 * W                # free size of one view-row block (2560)
CFW = C16 * FW
NVIEW = V - 1
WP = W + 2                  # padded variance width

_cache = {}
_GEOM = None                # (ncols tuple, runs) set by _host_prep

# engine split (v=DVE, g=Pool), tunable via env
_S1_ENG = os.environ.get("K_S1", "vvvv")   # per-view S1 x-interp
_S2_ENG = os.environ.get("K_S2", "vvvv")   # per-view S2 y-interp
_ACC_ENG = os.environ.get("K_ACC", "vv")   # vsum, vsq adds
_VAR_ENG = os.environ.get("K_VAR", "s")    # variance: 2=split, s=stt DVE
_TY_ENG = os.environ.get("K_TY", "a")      # tenty smalls: a=ACT, v=DVE
_VS_ENG = os.environ.get("K_VS", "p")      # vsum: p=PE psum, v/g=tt adds
_SQ_ENG = os.environ.get("K_SQ", "aaaaaa")  # squares: f0,4 views,m; a/v/g


def _build_program(nrep, geom):
    ncols, runs = geom
    nc = bacc.Bacc("TRN2", target_bir_lowering=False, debug=False,
                   num_devices=NCORES, num_swdge_queues=4)
    fwinall = nc.dram_tensor("fwinall", [SRC, 2, NVIEW * CFW], F16,
                             kind="ExternalInput")
    f0s = nc.dram_tensor("f0slab", [RH, 2, CJ], F16, kind="ExternalInput")
    a_all = nc.dram_tensor("a_all", [128, NVIEW * W], F32,
                           kind="ExternalInput")
    b1_all = nc.dram_tensor("b1_all", [128, NVIEW * W], F32,
                            kind="ExternalInput")
    txh = nc.dram_tensor("txall", [128, sum(ncols) * W], F16,
                         kind="ExternalInput")
    hmaskh = nc.dram_tensor("hmask", [128, 2 * RH], F32, kind="ExternalInput")
    bandh = nc.dram_tensor("band", [9 * NQ, 128, 64], F16,
                           kind="ExternalInput")
    identh = nc.dram_tensor("ident", [128, 128], F16, kind="ExternalInput")
    outh = nc.dram_tensor("out", [ROWS, D, W], F32, kind="ExternalOutput")

    with tile.TileContext(nc) as tc:
        import contextlib
        with contextlib.ExitStack() as ctx:
            const_p = ctx.enter_context(tc.tile_pool(name="const", bufs=1))
            aA = const_p.tile([128, NVIEW * W], F32)
            nc.sync.dma_start(aA[:], a_all.ap())
            bB = const_p.tile([128, NVIEW * W], F32)
            nc.sync.dma_start(bB[:], b1_all.ap())
            tx = const_p.tile([128, sum(ncols) * W], F16)
            nc.sync.dma_start(tx[:], txh.ap())
            hm = const_p.tile([128, 2 * RH], F32)
            nc.sync.dma_start(hm[:], hmaskh.ap())
            band = const_p.tile([128, 9 * NQ * 64], F16)
            nc.sync.dma_start(
                band[:],
                bass.AP(bandh, 0, [[64, 128], [8192, 9 * NQ], [1, 64]]))
            ident = const_p.tile([128, 128], F16)
            nc.sync.dma_start(ident[:], identh.ap())

            for rep in range(nrep):
                if rep > 0:
                    tc.strict_bb_all_engine_barrier()
                _emit_main(tc, nc, ncols, runs, fwinall, f0s, aA, bB, tx, hm,
                           band, ident, outh)
    nc.compile()
    return nc


def _eng(nc, ch):
    return nc.vector if ch == "v" else nc.gpsimd


_DMAQ_MAP = {"a": "scalar", "s": "sync", "p": "gpsimd"}


def _dmaq(nc, ch):
    return getattr(nc, _DMAQ_MAP[ch])


def _FR_DMA(nc):
    return _dmaq(nc, os.environ.get("K_FRQ", "s")[0])


def _F0_DMA(nc):
    return _dmaq(nc, os.environ.get("K_F0Q", "s")[0])


def _OUT_DMA(nc):
    return _dmaq(nc, os.environ.get("K_OUTQ", "s")[0])


def _emit_main(tc, nc, ncols, runs, fwinall, f0s, aA, bB, tx, hm, band,
               ident, outh):
    import contextlib
    txoff = [sum(ncols[:i]) * W for i in range(NVIEW)]
    with contextlib.ExitStack() as st:
        gx_p = st.enter_context(tc.tile_pool(name="gx", bufs=1))
        frep_p = st.enter_context(tc.tile_pool(name="frep", bufs=2))
        s1_p = st.enter_context(tc.tile_pool(name="s1", bufs=2))
        s2_p = st.enter_context(tc.tile_pool(name="s2", bufs=2))
        row_p = st.enter_context(tc.tile_pool(name="row", bufs=2))
        rw2_p = st.enter_context(tc.tile_pool(name="rw2", bufs=2))
        ty_p = st.enter_context(tc.tile_pool(name="ty", bufs=1))
        typ2 = ty_p
        m_p = st.enter_context(tc.tile_pool(name="msq", bufs=1))
        vr_p = st.enter_context(tc.tile_pool(name="vr", bufs=1))
        pp = st.enter_context(tc.tile_pool(name="cpsum", bufs=1, space="PSUM"))
        sp = st.enter_context(tc.tile_pool(name="soft", bufs=2))

        gslot, vslot = [], []
        for s in range(3):
            gs = gx_p.tile([128, NVIEW * CJ], F16, tag=f"gs{s}")
            gslot.append(gs)
            vs = vr_p.tile([128, C16 * WP], F16, tag=f"vr{s}")
            # zero the x-pad columns once; rows only ever write [1:W+1]
            nc.vector.memset(
                vs[:].rearrange("p (c x) -> p c x", x=WP)[:, :, 0:WP:WP - 1],
                0.0)
            vslot.append(vs)

        pv_p = st.enter_context(tc.tile_pool(name="vpsum", bufs=1,
                                             space="PSUM"))
        identf = ident
        ones64 = gx_p.tile([64, 64], F16, tag="ones64")
        nc.vector.memset(ones64[:], 1.0)

        def s1(ssi):
            # x-interp source row ssi (global Y = base-2+ssi) for all views
            s = ssi % 3
            fr = frep_p.tile([128, NVIEW * CFW], F16, tag="frall")
            nsp = int(os.environ.get("K_FRSPLIT", "1"))
            seg = NVIEW * CFW // nsp
            for si in range(nsp):
                in_ap = bass.AP(fwinall, ssi * (NVIEW * 2 * CFW) + si * seg,
                                [[NVIEW * CFW, 2], [0, 64], [1, seg]])
                _FR_DMA(nc).dma_start(fr[:, si * seg:(si + 1) * seg], in_ap)
            frv = fr[:].rearrange("p (v c w) -> p v c w", c=C16, w=FW)
            for vi in range(NVIEW):
                gsub = (gslot[s][:, vi * CJ:(vi + 1) * CJ]
                        .rearrange("p (c x) -> p c x", x=W))
                E = _eng(nc, _S1_ENG[vi])
                first = True
                for pos in range(ncols[vi]):
                    orig_i, rlist = runs[vi][pos]
                    for (xa, xb) in rlist:
                        txv = (tx[:, txoff[vi] + pos * W + xa:
                                  txoff[vi] + pos * W + xb]
                               .unsqueeze(1).broadcast_to([128, C16, xb - xa]))
                        fseg = frv[:, vi, :, orig_i + xa:orig_i + xb]
                        if first:
                            assert (xa, xb) == (0, W), \
                                "first S1 tap must cover full width"
                            E.tensor_tensor(gsub, txv, fseg, op=OP.mult)
                            first = False
                        else:
                            tm = s1_p.tile([128, CJ], F16, tag="tmpS1")
                            tmv = (tm[:].rearrange("p (c x) -> p c x", x=W)
                                   [:, :, xa:xb])
                            E.tensor_tensor(tmv, txv, fseg, op=OP.mult)
                            E.tensor_tensor(gsub[:, :, xa:xb],
                                            gsub[:, :, xa:xb], tmv, op=OP.add)

        def conv_row(ro):
            cost = pp.tile([64, W], F32, tag="cost")
            first = True
            for dy in range(3):
                vsv = (vslot[(ro + dy - 1) % 3][:]
                       .rearrange("p (c x) -> p c x", x=WP))
                for dx in range(3):
                    t = dy * 3 + dx
                    for k in range(NQ):
                        rhs = vsv[:, k, dx:dx + W]
                        lhsT = band[:, (t * NQ + k) * 64:(t * NQ + k + 1) * 64]
                        last = (dy == 2 and dx == 2 and k == NQ - 1)
                        nc.tensor.matmul(cost[:], lhsT, rhs,
                                         start=first, stop=last)
                        first = False
            # transpose-free softmax over d (on partitions): |cost| is small
            # (conv of variance with ~0.05-scale weights), so exp needs no
            # max subtraction; depth-sum via all-ones PE matmul replicates
            # the denominator across all 64 partitions.  The tail (sum,
            # reciprocal, scale, store) is batched over row pairs to halve
            # the instruction and DMA-issue count.
            e = sp.tile([64, W], F16, tag="e")
            nc.scalar.activation(e[:], cost[:], ACT.Exp)
            ssum = pp.tile([64, W], F32, tag="ssum")
            nc.tensor.matmul(ssum[:], ones64[:], e[:], start=True, stop=True)
            rinv = sp.tile([64, W], F16, tag="rinv")
            with nc.allow_low_precision(reason="softmax denom fp16 is ample"):
                nc.vector.reciprocal(rinv[:], ssum[:])
            prob = sp.tile([64, W], F32, tag="prob")
            nc.vector.tensor_tensor(prob[:], e[:], rinv[:], op=OP.mult)
            out_ap = bass.AP(outh, (ro - 1) * D * W, [[W, 64], [1, W]])
            _OUT_DMA(nc).dma_start(out_ap, prob[:])

        def emit_ty(r):
            # tenty weights for the 3 source rows (masked for halo rows)
            pyr = ty_p.tile([128, NVIEW * W], F32, tag="pyr")
            nc.vector.scalar_tensor_tensor(pyr[:], bB[:], float(r), aA[:],
                                           op0=OP.mult, op1=OP.add)
            hmc = hm[:, r:r + 1]
            hmn = hm[:, RH + r:RH + r + 1]
            ty0 = typ2.tile([128, NVIEW * W], F16, tag="ty0")
            ty1 = typ2.tile([128, NVIEW * W], F16, tag="ty1")
            ty2 = typ2.tile([128, NVIEW * W], F16, tag="ty2")
            if _TY_ENG == "a":
                # masked tents on ACT: Relu(scale*x + bias), per-part scale
                nc.scalar.activation(ty0[:], pyr[:], ACT.Relu, scale=hmn)
                nc.scalar.activation(ty2[:], pyr[:], ACT.Relu, scale=hmc)
                ab = ty_p.tile([128, NVIEW * W], F16, tag="ng")
                nc.scalar.activation(ab[:], pyr[:], ACT.Abs)
                nc.scalar.activation(ty1[:], ab[:], ACT.Relu,
                                     scale=hmn, bias=hmc)
            else:
                nc.vector.tensor_scalar(ty0[:], pyr[:], hmn, 0.0,
                                        op0=OP.mult, op1=OP.max)
                nc.vector.tensor_scalar(ty2[:], pyr[:], hmc, 0.0,
                                        op0=OP.mult, op1=OP.max)
                ng = ty_p.tile([128, NVIEW * W], F32, tag="ng")
                nc.vector.tensor_scalar(ng[:], pyr[:], -1.0, None,
                                        op0=OP.mult)
                nc.vector.tensor_tensor(ng[:], pyr[:], ng[:], op=OP.max)
                nc.vector.tensor_scalar(ng[:], ng[:], -1.0, 1.0,
                                        op0=OP.mult, op1=OP.add)
                nc.vector.tensor_scalar(ty1[:], ng[:], 0.0, hmc,
                                        op0=OP.max, op1=OP.mult)
            return (ty0, ty1, ty2)

        s1(0)
        s1(1)
        for r in range(RH):
            s1(r + 2)
            tys = emit_ty(r)

            # squares pre-scaled by sqrt(V) on ACT: Square(sqrt(V)*w) = V*w^2,
            # so the variance step is a plain 2x-mode subtract (no stt).
            sqscale = all(ch == "a" for ch in _SQ_ENG[:5])
            rtv = float(np.sqrt(V)) if sqscale else 1.0
            f0row = rw2_p.tile([128, CJ], F16, tag="f0row")
            _F0_DMA(nc).dma_start(
                f0row[:], bass.AP(f0s, r * (2 * CJ),
                                  [[CJ, 2], [0, 64], [1, CJ]]))
            vsq = row_p.tile([128, CJ], F16, tag="vsq")
            if _SQ_ENG[0] == "a":
                nc.scalar.activation(vsq[:], f0row[:], ACT.Square, scale=rtv)
            else:
                _eng(nc, _SQ_ENG[0]).tensor_tensor(vsq[:], f0row[:],
                                                   f0row[:], op=OP.mult)

            if _VS_ENG == "p":
                vs_ps = pv_p.tile([128, CJ], F32, tag="vsps")
                for ck in range(0, CJ, 512):
                    nc.tensor.matmul(vs_ps[:, ck:ck + 512], identf[:],
                                     f0row[:, ck:ck + 512],
                                     start=True, stop=False)
            else:
                vsum = row_p.tile([128, CJ], F16, tag="vsum")
            ea0 = _eng(nc, _ACC_ENG[0])
            ea1 = _eng(nc, _ACC_ENG[1])
            for vi in range(NVIEW):
                E = _eng(nc, _S2_ENG[vi])
                wv = s2_p.tile([128, CJ], F16, tag="warped")
                wvv = wv[:].rearrange("p (c x) -> p c x", x=W)
                for jj in range(3):
                    g = (gslot[(r + jj) % 3][:, vi * CJ:(vi + 1) * CJ]
                         .rearrange("p (c x) -> p c x", x=W))
                    t = (tys[jj][:, vi * W:(vi + 1) * W]
                         .unsqueeze(1).broadcast_to([128, C16, W]))
                    if jj == 0:
                        E.tensor_tensor(wvv, t, g, op=OP.mult)
                    else:
                        tw = s1_p.tile([128, CJ], F16, tag="tmpS1")
                        twv = tw[:].rearrange("p (c x) -> p c x", x=W)
                        E.tensor_tensor(twv, t, g, op=OP.mult)
                        E.tensor_tensor(wvv, wvv, twv, op=OP.add)
                if _VS_ENG == "p":
                    for ck in range(0, CJ, 512):
                        nc.tensor.matmul(vs_ps[:, ck:ck + 512], identf[:],
                                         wv[:, ck:ck + 512], start=False,
                                         stop=(vi == NVIEW - 1))
                elif vi == 0:
                    ea0.tensor_tensor(vsum[:], f0row[:], wv[:], op=OP.add)
                else:
                    ea0.tensor_tensor(vsum[:], vsum[:], wv[:], op=OP.add)
                sqv = rw2_p.tile([128, CJ], F16, tag="sqv")
                if _SQ_ENG[1 + vi] == "a":
                    nc.scalar.activation(sqv[:], wv[:], ACT.Square, scale=rtv)
                else:
                    _eng(nc, _SQ_ENG[1 + vi]).tensor_tensor(
                        sqv[:], wv[:], wv[:], op=OP.mult)
                ea1.tensor_tensor(vsq[:], vsq[:], sqv[:], op=OP.add)

            # variance scaled by V^2 (host folds 1/V^2 into the conv band):
            # varr = V*vsq - vsum^2
            m = m_p.tile([128, CJ], F16, tag="m")
            msrc = vs_ps if _VS_ENG == "p" else vsum
            if _SQ_ENG[5] == "a":
                nc.scalar.activation(m[:], msrc[:], ACT.Square)
            else:
                _eng(nc, _SQ_ENG[5]).tensor_tensor(m[:], msrc[:], msrc[:],
                                                   op=OP.mult)
            varr = (vslot[r % 3][:].rearrange("p (c x) -> p c x", x=WP)
                    [:, :, 1:W + 1])
            if sqscale:
                nc.vector.tensor_tensor(
                    varr, vsq[:].rearrange("p (c x) -> p c x", x=W),
                    m[:].rearrange("p (c x) -> p c x", x=W), op=OP.subtract)
            elif _VAR_ENG in ("2", "v"):
                tv = s1_p.tile([128, CJ], F16, tag="tmpS1")
                nc.vector.tensor_scalar(tv[:], vsq[:], float(V), None,
                                        op0=OP.mult)
                sube = nc.gpsimd if _VAR_ENG == "2" else nc.vector
                sube.tensor_tensor(
                    varr, tv[:].rearrange("p (c x) -> p c x", x=W),
                    m[:].rearrange("p (c x) -> p c x", x=W), op=OP.subtract)
            else:
                nc.vector.scalar_tensor_tensor(
                    varr, vsq[:].rearrange("p (c x) -> p c x", x=W),
                    float(V), m[:].rearrange("p (c x) -> p c x", x=W),
                    op0=OP.mult, op1=OP.subtract)
            if r >= 2:
                conv_row(r - 1)


def _get_runner(nrep=1):
    key = (nrep, _GEOM, _S1_ENG, _S2_ENG, _ACC_ENG, _VAR_ENG, _TY_ENG,
           _VS_ENG, _SQ_ENG,
           tuple(os.environ.get(k, "") for k in
                 ("K_FRQ", "K_F0Q", "K_OUTQ", "K_FRSPLIT")))
    if key in _cache:
        return _cache[key]
    import jax
    from jax.sharding import Mesh, PartitionSpec
    from jax.experimental.shard_map import shard_map
    from concourse.bass2jax import (_bass_exec_p, install_neuronx_cc_hook,
                                    partition_id_tensor)

    nc = _build_program(nrep, _GEOM)
    install_neuronx_cc_hook()
    partition_name = (nc.partition_id_tensor.name
                      if nc.partition_id_tensor else None)
    in_names, out_names, out_avals, zero_outs = [], [], [], []
    for alloc in nc.m.functions[0].allocations:
        if not isinstance(alloc, mybir.MemoryLocationSet):
            continue
        name = alloc.memorylocations[0].name
        if alloc.kind == "ExternalInput":
            if name != partition_name:
                in_names.append(name)
        elif alloc.kind == "ExternalOutput":
            shape = tuple(alloc.tensor_shape)
            dtype = mybir.dt.np(alloc.dtype)
            out_names.append(name)
            out_avals.append(jax.core.ShapedArray(shape, dtype))
            zero_outs.append(np.zeros(shape, dtype))
    n_params, n_outs = len(in_names), len(out_avals)
    all_in = list(in_names) + list(out_names) + (
        [partition_name] if partition_name else [])

    def _body(*args):
        operands = list(args)
        if partition_name is not None:
            operands.append(partition_id_tensor())
        outs = _bass_exec_p.bind(
            *operands, out_avals=tuple(out_avals), in_names=tuple(all_in),
            out_names=tuple(out_names), lowering_input_output_aliases=(),
            sim_require_finite=True, sim_require_nnan=True, nc=nc)
        return tuple(outs)

    devices = jax.devices()[:NCORES]
    mesh = Mesh(np.asarray(devices), ("core",))
    in_specs = (PartitionSpec("core"),) * (n_params + n_outs)
    out_specs = (PartitionSpec("core"),) * n_outs
    sharded = jax.jit(
        shard_map(_body, mesh=mesh, in_specs=in_specs, out_specs=out_specs,
                  check_rep=False), keep_unused=True)

    from jax.sharding import NamedSharding
    shard = NamedSharding(mesh, PartitionSpec("core"))
    dev_cache = {}

    def run(in_maps, fetch=True):
        ck = id(in_maps)
        if ck not in dev_cache:
            per_core = [[np.asarray(m[n]) for n in in_names] for m in in_maps]
            concat_in = [
                np.concatenate([per_core[c][i] for c in range(NCORES)], axis=0)
                for i in range(n_params)]
            concat_zeros = [
                np.zeros((NCORES * z.shape[0], *z.shape[1:]), z.dtype)
                for z in zero_outs]
            dev_cache.clear()
            dev_cache[ck] = [jax.device_put(x, shard)
                             for x in concat_in + concat_zeros]
            jax.block_until_ready(dev_cache[ck])
        out_arrs = sharded(*dev_cache[ck])
        jax.block_until_ready(out_arrs)
        if not fetch:
            return None
        return [{n: np.asarray(out_arrs[i]).reshape(
                    NCORES, *out_avals[i].shape)[c]
                 for i, n in enumerate(out_names)} for c in range(NCORES)]

    _cache[key] = run
    return run


def _host_prep(feat0, feat1, feat2, feat3, feat4, proj_matrices, depth_values,
               conv_w):
    global _GEOM
    feats = [np.asarray(f, np.float32) for f in
             (feat0, feat1, feat2, feat3, feat4)]
    projs = np.asarray(proj_matrices, np.float64)
    depth = np.asarray(depth_values, np.float64)[0]          # [D]
    w3 = np.asarray(conv_w, np.float32)[0]                   # [C,3,3,3]

    def fuse(p):  # p [2,4,4]
        out = p[0].copy()
        out[:3, :4] = p[1, :3, :3] @ p[0, :3, :4]
        return out

    ref = fuse(projs[0, 0])
    ref_inv = np.linalg.inv(ref)
    Rs, ts = [], []
    for v in range(1, V):
        P = fuse(projs[0, v]) @ ref_inv
        Rs.append(P[:3, :3])
        ts.append(P[:3, 3])
        assert abs(P[0, 1]) < 1e-5 and abs(P[2, 1]) < 1e-5, "px depends on y"
        assert abs(P[1, 1] - 1.0) < 1e-5, "py y-slope != 1"

    # per-view window geometry (shared by all cores); p = (cl, d)
    dgrid = np.arange(128) % 64
    dep = depth[dgrid]                                       # [128]
    xg = np.arange(W, dtype=np.float64)[None, :]             # [1, W]

    sxs, ncols_l, pxs = [], [], []
    for v in range(1, V):
        R, t = Rs[v - 1], ts[v - 1]
        den = (R[2, 0] * xg + R[2, 2]) * dep[:, None] + t[2]
        px = ((R[0, 0] * xg + R[0, 2]) * dep[:, None] + t[0]) / den
        rel = px - xg
        sx = int(np.floor(rel.min()))
        nc_ = int(np.floor(rel.max())) + 2 - sx
        assert 2 <= nc_ <= 3, f"view {v}: ncols={nc_}"
        assert nc_ - 1 + W <= FW, "window fits"
        sxs.append(sx)
        ncols_l.append(nc_)
        pxs.append(px)

    # tent weights for x + active-x runs per (view, tap)
    txall = np.zeros((128, sum(ncols_l) * W), np.float16)
    runs_l = []
    off = 0
    for vi in range(NVIEW):
        fx = pxs[vi] - xg - sxs[vi]
        nc_ = ncols_l[vi]
        if nc_ == 2:
            assert fx.min() > 0 and fx.max() < 1
            tents = [1.0 - fx, fx]
        else:
            assert fx.min() > 0 and fx.max() < 2
            tents = [np.maximum(0.0, 1.0 - fx),
                     1.0 - np.abs(fx - 1.0),
                     np.maximum(0.0, fx - 1.0)]
        # order taps so a full-width tap comes first (it is the writer)
        vruns = []
        for i, tn in enumerate(tents):
            txall[:, off + i * W: off + (i + 1) * W] = tn.astype(np.float16)
            active = (tn > 0).any(axis=0)                    # [W]
            if active.all():
                vruns.append(((0, W),))
            else:
                # contiguous runs of active columns
                idx = np.flatnonzero(active)
                assert len(idx) > 0
                brk = np.flatnonzero(np.diff(idx) > 1)
                starts = np.concatenate([[idx[0]], idx[brk + 1]])
                ends = np.concatenate([idx[brk] + 1, [idx[-1] + 1]])
                vruns.append(tuple((int(a), int(b))
                                   for a, b in zip(starts, ends)))
        order = sorted(range(nc_), key=lambda i: vruns[i] != ((0, W),))
        assert vruns[order[0]] == ((0, W),), f"view {vi + 1}: no full tap"
        # reorder tents in txall to match emission order
        tx2 = txall[:, off:off + nc_ * W].copy()
        vr2 = []
        for pos, i in enumerate(order):
            txall[:, off + pos * W: off + (pos + 1) * W] = \
                tx2[:, i * W:(i + 1) * W]
            vr2.append(vruns[i])
        # emission reads fr columns [i + xa, i + xb): keep tap index mapping
        runs_l.append(tuple((order[pos], tuple(vr2[pos]))
                            for pos in range(nc_)))
        off += nc_ * W
    _GEOM = (tuple(ncols_l), tuple(runs_l))

    # staged window tensor: fwinall[Y+2, cl, v, c16*FW] (channel ch=2*c16+cl)
    fwin_full = np.zeros((H + 4, 2, NVIEW, C16, FW), np.float16)
    for v in range(1, V):
        fpad = np.zeros((H + 4, C, 256), np.float16)
        fpad[2:H + 2, :, :W] = feats[v][0].transpose(1, 0, 2)
        lo = sxs[v - 1]
        for cl in range(2):
            fwin_full[:, cl, v - 1] = fpad[:, cl::2, lo:lo + FW]

    # conv band matrices (shared), fp16, with 1/V^2 folded in
    band = np.zeros((9, NQ, 128, 64), np.float32)
    d_ = np.arange(64)
    dz = d_[:, None] - d_[None, :] + 1
    msk = (dz >= 0) & (dz < 3)
    dzc = np.clip(dz, 0, 2)
    for dy in range(3):
        for dx in range(3):
            for k in range(NQ):
                for cl in range(2):
                    c = 2 * k + cl
                    blk = np.where(msk, w3[c, dzc, dy, dx] / (V * V), 0.0)
                    band[dy * 3 + dx, k, cl * 64:(cl + 1) * 64, :] = blk
    band = band.reshape(9 * NQ, 128, 64).astype(np.float16)
    ident = np.eye(128, dtype=np.float16)

    f0pad = np.zeros((H + 2, C, W), np.float32)
    f0pad[1:H + 1] = feats[0][0].transpose(1, 0, 2)

    in_maps = []
    for core in range(NCORES):
        base = core * ROWS
        # y-interp coefficients: pyr = A'' + r*B1 per (p, view, x)
        aall = np.zeros((128, NVIEW * W), np.float32)
        ball = np.zeros((128, NVIEW * W), np.float32)
        for vi in range(NVIEW):
            R, t = Rs[vi], ts[vi]
            den = (R[2, 0] * xg + R[2, 2]) * dep[:, None] + t[2]
            rd = 1.0 / den
            ny0 = (R[1, 0] * xg + R[1, 2] + (base - 1)) * dep[:, None] + t[1]
            a2 = ny0 * rd - (base - 1)
            b1 = dep[:, None] * rd - 1.0
            aall[:, vi * W:(vi + 1) * W] = a2
            ball[:, vi * W:(vi + 1) * W] = b1
        # halo masks: rows outside [0, H)
        hmask = np.zeros((128, 2 * RH), np.float32)
        for r in range(RH):
            y = base - 1 + r
            mval = 1.0 if 0 <= y < H else 0.0
            hmask[:, r] = mval
            hmask[:, RH + r] = -mval
        # ref feature slab rows base-1 .. base+16, channel-parity split
        f0slab = np.zeros((RH, 2, C16, W), np.float16)
        for r in range(RH):
            for cl in range(2):
                f0slab[r, cl] = (f0pad[base + r, cl::2, :]
                                 .astype(np.float16))
        m = dict(fwinall=fwin_full[base:base + SRC]
                 .reshape(SRC, 2, NVIEW * CFW).copy(),
                 f0slab=f0slab.reshape(RH, 2, CJ), a_all=aall, b1_all=ball,
                 txall=txall, hmask=hmask, band=band, ident=ident)
        in_maps.append(m)
    return in_maps


def kernel(feat0, feat1, feat2, feat3, feat4, proj_matrices, depth_values,
           num_depth=None, conv_w=None, conv_b=None, **_):
    in_maps = _host_prep(feat0, feat1, feat2, feat3, feat4, proj_matrices,
                         depth_values, conv_w)
    run = _get_runner(1)
    res = run(in_maps)
    out = np.zeros((B, D, H, W), np.float32)
    for core in range(NCORES):
        o = res[core]["out"]                                 # [ROWS, D, W]
        out[0, :, core * ROWS:(core + 1) * ROWS, :] = o.transpose(1, 0, 2)
    return out
